# revision 21
# baseline (speedup 1.0000x reference)
"""Trainium2 Bass kernel for nn_AEEncoder: 256 independent per-TF blocks
(gene->hidden->hidden->TF-activity) with BatchNorm+LeakyReLU between layers.

Sharding: expert-parallel over the TF axis. Each of the 8 cores owns 32 TFs
(a contiguous 4096-column slice of `features`) and the full batch, so all
three BatchNorms are core-local (stats are per-feature over the batch) and
no collectives are needed. Host assembles the [4096, 256] output from the
per-core [32, 4096] TF-major outputs.

Biases b1/b2/b3 cancel under BatchNorm (BN subtracts the mean), so they are
accepted but unused.

On-chip dataflow is feature-major ([feature partitions, batch free]):
  - features are DMA'd batch-major with an f32->bf16 cast (SWDGE), then
    flipped feature-major with PE transposes (bf16, via identity).
  - L1: per-TF [128g x 64k] matmul, a TF pair col-tiled into one PSUM bank.
  - L2: pair-block-diagonal [128 x 128] stationary, one matmul per chunk.
  - L3: pair [128 x 2] stationary, two pairs col-tiled at partitions 0/32.
  - BN: DVE bn_stats/bn_aggr per 512-col PSUM chunk; rsqrt via DVE
    reciprocal + ACT Sqrt; BN-apply + LeakyReLU fused into one ACT Prelu
    (per-partition scale/bias, alpha=0.01).
"""
import sys

sys.path.insert(0, "/opt/trn_rl_repo")

import numpy as np
import ml_dtypes

from concourse import bacc, bass, mybir, tile
from concourse.bass_utils import run_bass_kernel_spmd

F32 = mybir.dt.float32
BF16 = mybir.dt.bfloat16
AF = mybir.ActivationFunctionType
ALU = mybir.AluOpType

B = 4096           # batch
T_CORE = 32        # TFs per core
G = 128            # genes per TF
K = 64             # hidden nodes per TF
N_CORES = 8
EPS = 1e-5
ALPHA = 0.01       # LeakyReLU negative slope

NPAIR = T_CORE // 2          # 16 TF pairs per core
NCH = B // 512               # 8 batch chunks of 512
NBT = B // 128               # 32 batch tiles of 128 per TF


def _build():
    nc = bacc.Bacc("TRN2", target_bir_lowering=False, debug=False, num_devices=N_CORES)

    feat = nc.declare_dram_parameter("features", [B, T_CORE * G], F32, isOutput=False)
    w1 = nc.declare_dram_parameter("w1t", [128, NPAIR * 2 * K], BF16, isOutput=False)
    w2 = nc.declare_dram_parameter("w2bd", [128, NPAIR * 128], BF16, isOutput=False)
    w3 = nc.declare_dram_parameter("w3bd", [128, NPAIR * 2], BF16, isOutput=False)
    ident_d = nc.declare_dram_parameter("ident", [128, 128], BF16, isOutput=False)
    zout = nc.declare_dram_parameter("zout", [T_CORE, B], F32, isOutput=True)

    XCOPY_ACT = 4   # of every 8 transpose-bank copies, this many go to ACT

    with tile.TileContext(nc) as tc:
        with (
            tc.tile_pool(name="const", bufs=1) as constp,
            tc.tile_pool(name="xbm", bufs=8) as xbmp,
            tc.tile_pool(name="xfm", bufs=4) as xfmp,
            tc.tile_pool(name="h1hat", bufs=3) as h1hatp,
            tc.tile_pool(name="h2hat", bufs=5) as h2hatp,
            tc.tile_pool(name="zhat", bufs=1) as zhatp,
            tc.tile_pool(name="stats", bufs=4) as statsp,
            tc.tile_pool(name="ps_tp", bufs=2, space="PSUM") as pstp,
            tc.tile_pool(name="ps_p1", bufs=2, space="PSUM") as psp1,
            tc.tile_pool(name="ps_p2", bufs=2, space="PSUM") as psp2,
            tc.tile_pool(name="ps_z", bufs=2, space="PSUM") as psz,
        ):
            w1_sb = constp.tile([128, NPAIR * 2 * K], BF16)
            nc.sync.dma_start(out=w1_sb[:], in_=w1[:])
            w2_sb = constp.tile([128, NPAIR * 128], BF16)
            nc.sync.dma_start(out=w2_sb[:], in_=w2[:])
            w3_sb = constp.tile([128, NPAIR * 2], BF16)
            nc.sync.dma_start(out=w3_sb[:], in_=w3[:])
            ident = constp.tile([128, 128], BF16)
            nc.sync.dma_start(out=ident[:], in_=ident_d[:])

            def bn_prep(st, tag):
                """st [128, NCH*6] bn_stats chunks -> (s, beta) f32 [128,1]."""
                mv = statsp.tile([128, 2], F32, tag=f"mv{tag}")
                nc.vector.bn_aggr(mv[:], st[:])
                var = statsp.tile([128, 1], F32, tag=f"var{tag}")
                nc.vector.tensor_scalar(var[:], mv[:, 1:2], EPS, None, ALU.add)
                inv = statsp.tile([128, 1], F32, tag=f"inv{tag}")
                nc.vector.reciprocal(inv[:], var[:])
                s = statsp.tile([128, 1], F32, tag=f"s{tag}")
                nc.scalar.activation(s[:], inv[:], AF.Sqrt)
                beta = statsp.tile([128, 1], F32, tag=f"b{tag}")
                nc.vector.tensor_scalar(beta[:], mv[:, 0:1], -1.0, None, ALU.mult)
                nc.vector.tensor_tensor(beta[:], beta[:], s[:], ALU.mult)
                return s, beta

            def mm_l1(hp, l1w, xfm_t, c):
                nc.tensor.matmul(
                    hp[0:64, :], l1w[0], xfm_t[0][:, c * 512:(c + 1) * 512],
                    start=True, stop=True,
                )
                nc.tensor.matmul(
                    hp[64:128, :], l1w[1], xfm_t[1][:, c * 512:(c + 1) * 512],
                    start=True, stop=True, tile_position=(0, 64),
                )

            def emit_load(p):
                xbm_t = []
                for t in (2 * p, 2 * p + 1):
                    xbm = xbmp.tile([128, B], BF16, tag="xbm")
                    srcap = feat[:, t * G:(t + 1) * G].rearrange(
                        "(j p) g -> p j g", p=128
                    )
                    nc.gpsimd.dma_start(out=xbm[:], in_=srcap)
                    xbm_t.append(xbm)
                return xbm_t

            def l1w_of(p):
                return [
                    w1_sb[:, (2 * p + 0) * K:(2 * p + 1) * K],
                    w1_sb[:, (2 * p + 1) * K:(2 * p + 2) * K],
                ]

            class Pair:
                """Emit-on-demand phases for one TF pair; each phase emits
                one 512-batch chunk c in 0..7."""

                def __init__(self, p):
                    self.p = p
                    self.xbm_t = emit_load(p)
                    self.xfm_t = [
                        xfmp.tile([128, B], BF16, tag="xfm", name="xfm") for _ in range(2)
                    ]
                    self.st1 = statsp.tile([128, NCH * 6], F32, tag="st1")
                    self.st2 = statsp.tile([128, NCH * 6], F32, tag="st2")
                    self.h1 = h1hatp.tile([128, B], BF16, tag="h1hat")
                    self.h2 = h2hatp.tile([128, B], BF16, tag="h2hat")

                def T(self, c):
                    # two copy-groups (q = 2c, 2c+1), covering all 16 per pair
                    for q in (2 * c, 2 * c + 1):
                        e, qq = divmod(q, NBT // 4)
                        tp = pstp.tile([128, 512], BF16, tag="tp")
                        for m in range(4):
                            j = 4 * qq + m
                            nc.tensor.transpose(
                                tp[:, m * 128:(m + 1) * 128],
                                self.xbm_t[e][:, j * 128:(j + 1) * 128],
                                ident[:],
                            )
                        dst = self.xfm_t[e][:, qq * 512:(qq + 1) * 512]
                        if qq < XCOPY_ACT:
                            nc.scalar.copy(dst, tp[:])
                        else:
                            nc.vector.tensor_copy(dst, tp[:])

                def P1(self, c):
                    hp = psp1.tile([128, 512], F32, tag="hp1")
                    mm_l1(hp, l1w_of(self.p), self.xfm_t, c)
                    nc.vector.bn_stats(self.st1[:, c * 6:(c + 1) * 6], hp[:])

                def P2(self, c):
                    if c == 0:
                        self.sb1 = bn_prep(self.st1, "1")
                    hp = psp2.tile([128, 512], F32, tag="hp2")
                    mm_l1(hp, l1w_of(self.p), self.xfm_t, c)
                    nc.scalar.activation(
                        self.h1[:, c * 512:(c + 1) * 512], hp[:], AF.Prelu,
                        bias=self.sb1[1][:], scale=self.sb1[0][:], alpha=ALPHA,
                    )

                def mm_l2(self, hp, c):
                    nc.tensor.matmul(
                        hp[:], w2_sb[:, self.p * 128:(self.p + 1) * 128],
                        self.h1[:, c * 512:(c + 1) * 512],
                        start=True, stop=True,
                    )

                def Q1(self, c):
                    hp = psp1.tile([128, 512], F32, tag="hp1")
                    self.mm_l2(hp, c)
                    nc.vector.bn_stats(self.st2[:, c * 6:(c + 1) * 6], hp[:])

                def Q2(self, c):
                    if c == 0:
                        self.sb2 = bn_prep(self.st2, "2")
                    hp = psp2.tile([128, 512], F32, tag="hp2")
                    self.mm_l2(hp, c)
                    nc.scalar.activation(
                        self.h2[:, c * 512:(c + 1) * 512], hp[:], AF.Prelu,
                        bias=self.sb2[1][:], scale=self.sb2[0][:], alpha=ALPHA,
                    )

            class ZPhase:
                """L3 + BN3 for one subgroup (needs both pairs' h2)."""

                def __init__(self, sg, pA, pB):
                    self.sg = sg
                    self.pairs = (pA, pB)
                    self.st3 = statsp.tile([128, NCH * 6], F32, tag="st3")
                    self.zh = zhatp.tile([128, B], F32, tag="zhat")

                def mm(self, zp, c):
                    for i, pr in enumerate(self.pairs):
                        nc.tensor.matmul(
                            zp[32 * i:32 * i + 2, :],
                            w3_sb[:, pr.p * 2:(pr.p + 1) * 2],
                            pr.h2[:, c * 512:(c + 1) * 512],
                            start=True, stop=True,
                            tile_position=(0, 32 * i) if i else None,
                        )

                def Z1(self, c):
                    zp = psz.tile([128, 512], F32, tag="zps")
                    self.mm(zp, c)
                    nc.vector.bn_stats(self.st3[:, c * 6:(c + 1) * 6], zp[:])

                def Z2(self, c):
                    if c == 0:
                        self.sb3 = bn_prep(self.st3, "3")
                    zp = psz.tile([128, 512], F32, tag="zps")
                    self.mm(zp, c)
                    nc.scalar.activation(
                        self.zh[:, c * 512:(c + 1) * 512], zp[:], AF.Prelu,
                        bias=self.sb3[1][:], scale=self.sb3[0][:], alpha=ALPHA,
                    )
                    if c == NCH - 1:
                        for e in range(2):
                            nc.sync.dma_start(
                                out=zout[4 * self.sg + e:4 * self.sg + e + 3:2, :],
                                in_=self.zh[e:e + 33:32, :],
                            )

            def zipc(*phases):
                for c in range(NCH):
                    for ph in phases:
                        ph(c)

            # -------- software pipeline over 8 subgroups of 2 pairs --------
            NSG = NPAIR // 2
            prev_z = None
            A = Pair(0)
            zipc(A.T)
            for sg in range(NSG):
                Bp = Pair(2 * sg + 1)
                if prev_z is None:
                    zipc(A.P1, Bp.T)
                else:
                    zipc(prev_z.Z2, A.P1, Bp.T)
                zipc(A.P2, Bp.P1)
                zipc(A.Q1, Bp.P2)
                zipc(A.Q2, Bp.Q1)
                z = ZPhase(sg, A, Bp)
                if sg < NSG - 1:
                    nextA = Pair(2 * sg + 2)
                    zipc(Bp.Q2, z.Z1, nextA.T)
                    A = nextA
                else:
                    zipc(Bp.Q2, z.Z1)
                prev_z = z
            zipc(prev_z.Z2)

    nc.finalize()
    return nc


_NC = None


def _get_nc():
    global _NC
    if _NC is None:
        _NC = _build()
    return _NC


def _make_in_maps(features, W1, W2, W3):
    bf = ml_dtypes.bfloat16
    ident = np.eye(128, dtype=bf)
    in_maps = []
    for i in range(N_CORES):
        tfs = slice(i * T_CORE, (i + 1) * T_CORE)
        w1c = W1[tfs]                       # [32, 64, 128]
        w2c = W2[tfs]                       # [32, 64, 64]
        w3c = W3[tfs]                       # [32, 64]
        # w1t [128, pair, e, K]: lhsT per TF = W1[t].T  ([g, k])
        w1t = np.zeros((128, NPAIR, 2, K), dtype=bf)
        w1t[:, :, :, :] = (
            w1c.transpose(2, 0, 1).reshape(128, NPAIR, 2, K).astype(bf)
        )
        # w2bd [128, pair, 128]: blockdiag(W2[t0].T, W2[t1].T)
        w2bd = np.zeros((128, NPAIR, 128), dtype=np.float32)
        for pp in range(NPAIR):
            w2bd[0:64, pp, 0:64] = w2c[2 * pp].T
            w2bd[64:128, pp, 64:128] = w2c[2 * pp + 1].T
        # w3bd [128, pair, 2]
        w3bd = np.zeros((128, NPAIR, 2), dtype=np.float32)
        for pp in range(NPAIR):
            w3bd[0:64, pp, 0] = w3c[2 * pp]
            w3bd[64:128, pp, 1] = w3c[2 * pp + 1]
        in_maps.append({
            "features": np.ascontiguousarray(features[:, i * T_CORE * G:(i + 1) * T_CORE * G]),
            "w1t": np.ascontiguousarray(w1t.reshape(128, NPAIR * 2 * K)),
            "w2bd": np.ascontiguousarray(w2bd.reshape(128, NPAIR * 128).astype(bf)),
            "w3bd": np.ascontiguousarray(w3bd.reshape(128, NPAIR * 2).astype(bf)),
            "ident": ident,
        })
    return in_maps


def _run(in_maps, **kwargs):
    nc = _get_nc()
    return run_bass_kernel_spmd(nc, in_maps, core_ids=list(range(N_CORES)), **kwargs)


def kernel(features, W1, b1, W2, b2, W3, b3):
    features = np.asarray(features, dtype=np.float32)
    in_maps = _make_in_maps(
        features,
        np.asarray(W1, dtype=np.float32),
        np.asarray(W2, dtype=np.float32),
        np.asarray(W3, dtype=np.float32),
    )
    res = _run(in_maps)
    z = np.concatenate([r["zout"] for r in res.results], axis=0)  # [256, 4096]
    return np.ascontiguousarray(z.T).astype(np.float32)           # [4096, 256]
